# revision 6
# baseline (speedup 1.0000x reference)
"""Embedding lookup (gather) kernel for Trainium2, 8 NeuronCores.

Reference computes emb[b,s,:] = weight[x[b,s],:]. Data-parallel over the
B*S = 4096 tokens, 512 tokens per core. The [32000, 512] f32 table is
converted to bf16 on the host (rel err ~2^-8, far inside the 2e-2 gate),
halving HBM traffic in both directions versus the f32 v1 baseline.

v2: the four serialized 128-offset indirect_dma_start gathers (994ns fixed
SWDGE cost each, ~5.6us total on gpsimd) are replaced with ONE
InstDMAGatherAnt (dma_gather) covering all 512 rows: 994 + 0.34*512 ~ 1.2us
of descriptor generation. dma_gather needs the `mlp` GPSIMD ucode library
(load_library) and int16 indices wrapped into 16 partitions (idx i at
partition i%16, column i//16) replicated 8x across the Q7 core groups.
Both the library switch and a 128-row warmup gather (zeroed offsets) run
while the idx DMA is in flight.

dma_gather's output layout is exactly the j-major layout v1 used:
gathered row i -> partition i%128, chunk i//128, so out rows j*128+p hold
token j*128+p and the host-side unshard stays a plain reshape.
"""

import numpy as np

import concourse.bacc as bacc
import concourse.bass as bass
from concourse import mybir
from concourse.bass_utils import run_bass_kernel_spmd

B, S = 4, 1024
V, D = 32000, 512
N_CORES = 8
TOK = B * S                      # 4096 total tokens
TPC = TOK // N_CORES             # 512 tokens per core
P = 128                          # SBUF partitions
NCH = TPC // P                   # 4 j-slots of 128 rows
IDX_COLS = TPC // 16             # 32 int16 columns in the wrapped idx layout

_CACHE: dict = {}


def _build() -> bass.Bass:
    # Bacc (not raw Bass): its compile() pass auto-inserts and properly
    # lowers the MODIFY_POOL_CONFIG library switch that dma_gather needs
    # (raw Bass leaves InstPseudoReloadLibraryIndex unlowered and walrus
    # codegen rejects it with "ISA wrong length").
    nc = bacc.Bacc("TRN2")
    idx = nc.dram_tensor("idx", [P, IDX_COLS], mybir.dt.int16, kind="ExternalInput")
    w = nc.dram_tensor("weight", [V, D], mybir.dt.bfloat16, kind="ExternalInput")
    out = nc.dram_tensor("out", [TPC, D], mybir.dt.bfloat16, kind="ExternalOutput")
    with (
        nc.Block() as block,
        nc.semaphore("idx_sem") as idx_sem,
        nc.semaphore("g0") as g0,
        nc.semaphore("wm") as wm,
        nc.semaphore("wu") as wu,
        nc.semaphore("s0") as s0,
        nc.semaphore("s1") as s1,
        nc.sbuf_tensor("idx_t", [P, IDX_COLS], mybir.dt.int16) as idx_t,
        nc.sbuf_tensor("emb", [P, NCH, D], mybir.dt.bfloat16) as emb,
        nc.sbuf_tensor("woff", [P, 8], mybir.dt.int16) as woff,
        nc.sbuf_tensor("scr", [P, 1, D], mybir.dt.bfloat16) as scr,
    ):
        @block.sync
        def _(s):
            s.dma_start(out=idx_t[:], in_=idx[:]).then_inc(idx_sem, 16)

        @block.gpsimd
        def _(g):
            # pay dma_gather's first-use cost (library switch + IRAM page-in)
            # on a 128-row row-0 gather while the idx DMA is in flight;
            # Bacc.insert_library_loads places the mlp-library switch here
            g.memset(woff[:], 0).then_inc(wm, 1)
            g.wait_ge(wm, 1)
            g.dma_gather(scr[:], w[:], woff[:], P, P, D).then_inc(wu, 16)
            g.wait_ge(idx_sem, 16)
            g.dma_gather(emb[:], w[:], idx_t[:], TPC, TPC, D).then_inc(g0, 16)

        @block.sync
        def _(s):
            s.wait_ge(g0, 16)
            s.dma_start(out=out[0:P, :], in_=emb[:, 0, :]).then_inc(s0, 16)
            s.dma_start(out=out[3 * P : 4 * P, :], in_=emb[:, 3, :]).then_inc(s0, 16)

        @block.scalar
        def _(a):
            a.wait_ge(g0, 16)
            a.dma_start(out=out[P : 2 * P, :], in_=emb[:, 1, :]).then_inc(s1, 16)
            a.dma_start(out=out[2 * P : 3 * P, :], in_=emb[:, 2, :]).then_inc(s1, 16)
            # block-end DRAIN on each engine waits for its HWDGE queue
            # completion (verified exact on HW by the v1 baseline)

    return nc


def _pack_idx(flat_slice: np.ndarray) -> np.ndarray:
    """[TPC] int -> [128, 32] int16: idx i at (i%16, i//16), tiled 8x down
    the partitions for the 8 Q7 core groups."""
    blk = np.ascontiguousarray(flat_slice.astype(np.int16).reshape(IDX_COLS, 16).T)
    return np.ascontiguousarray(np.tile(blk, (8, 1)))


def kernel(x: np.ndarray, weight: np.ndarray) -> np.ndarray:
    import ml_dtypes

    x = np.asarray(x)
    flat = np.ascontiguousarray(x.reshape(-1)).astype(np.int64)
    w16 = np.ascontiguousarray(
        np.asarray(weight, dtype=np.float32).astype(ml_dtypes.bfloat16)
    )
    _CACHE["w16"] = w16  # test.py --profile reuses the converted table

    if "nc" not in _CACHE:
        nc = _build()
        # run_bass_via_pjrt binds the exec primitive directly and never
        # finalizes; Bacc needs finalize() -> compile() for register
        # allocation and the mlp-library MODIFY_POOL_CONFIG lowering
        nc.finalize()
        _CACHE["nc"] = nc
    nc = _CACHE["nc"]

    in_maps = [
        {
            "idx": _pack_idx(flat[i * TPC : (i + 1) * TPC]),
            "weight": w16,
        }
        for i in range(N_CORES)
    ]
    res = run_bass_kernel_spmd(nc, in_maps, list(range(N_CORES)))
    outs = [
        np.asarray(res.results[i]["out"]).astype(np.float32) for i in range(N_CORES)
    ]
    return np.concatenate(outs, axis=0).reshape(B, S, D)


# revision 7
# speedup vs baseline: 1.0315x; 1.0315x over previous
"""Embedding lookup (gather) kernel for Trainium2, 8 NeuronCores.

Reference computes emb[b,s,:] = weight[x[b,s],:]. Data-parallel over the
B*S = 4096 tokens, 512 tokens per core. The [32000, 512] f32 table is
converted to bf16 on the host (rel err ~2^-8, far inside the 2e-2 gate),
halving HBM traffic in both directions versus the f32 v1 baseline.

v2: the four serialized 128-offset indirect_dma_start gathers (994ns fixed
SWDGE cost each, ~5.6us total on gpsimd) are replaced with ONE
InstDMAGatherAnt (dma_gather) covering all 512 rows: 994 + 0.34*512 ~ 1.2us
of descriptor generation. dma_gather needs the `mlp` GPSIMD ucode library
(load_library) and int16 indices wrapped into 16 partitions (idx i at
partition i%16, column i//16) replicated 8x across the Q7 core groups.
Both the library switch and a 128-row warmup gather (zeroed offsets) run
while the idx DMA is in flight.

dma_gather's output layout is exactly the j-major layout v1 used:
gathered row i -> partition i%128, chunk i//128, so out rows j*128+p hold
token j*128+p and the host-side unshard stays a plain reshape.
"""

import numpy as np

import concourse.bacc as bacc
import concourse.bass as bass
from concourse import mybir
from concourse.bass_utils import run_bass_kernel_spmd

B, S = 4, 1024
V, D = 32000, 512
N_CORES = 8
TOK = B * S                      # 4096 total tokens
TPC = TOK // N_CORES             # 512 tokens per core
P = 128                          # SBUF partitions
NCH = TPC // P                   # 4 j-slots of 128 rows
IDX_COLS = TPC // 16             # 32 int16 columns in the wrapped idx layout

_CACHE: dict = {}


def _build() -> bass.Bass:
    # Bacc (not raw Bass): its compile() pass auto-inserts and properly
    # lowers the MODIFY_POOL_CONFIG library switch that dma_gather needs
    # (raw Bass leaves InstPseudoReloadLibraryIndex unlowered and walrus
    # codegen rejects it with "ISA wrong length").
    nc = bacc.Bacc("TRN2")
    idx = nc.dram_tensor("idx", [P, IDX_COLS], mybir.dt.int16, kind="ExternalInput")
    w = nc.dram_tensor("weight", [V, D], mybir.dt.bfloat16, kind="ExternalInput")
    out = nc.dram_tensor("out", [TPC, D], mybir.dt.bfloat16, kind="ExternalOutput")
    with (
        nc.Block() as block,
        nc.semaphore("idx_sem") as idx_sem,
        nc.semaphore("g0") as g0,
        nc.semaphore("wm") as wm,
        nc.semaphore("wu") as wu,
        nc.semaphore("s0") as s0,
        nc.semaphore("s1") as s1,
        nc.sbuf_tensor("idx_t", [P, IDX_COLS], mybir.dt.int16) as idx_t,
        nc.sbuf_tensor("emb", [P, NCH, D], mybir.dt.bfloat16) as emb,
        nc.sbuf_tensor("woff", [P, 8], mybir.dt.int16) as woff,
        nc.sbuf_tensor("scr", [P, 1, D], mybir.dt.bfloat16) as scr,
    ):
        @block.sync
        def _(s):
            s.dma_start(out=idx_t[:], in_=idx[:]).then_inc(idx_sem, 16)

        @block.gpsimd
        def _(g):
            # pay dma_gather's first-use cost (library switch + IRAM page-in)
            # on a 128-row row-0 gather while the idx DMA is in flight;
            # Bacc.insert_library_loads places the mlp-library switch here
            g.memset(woff[:], 0).then_inc(wm, 1)
            g.wait_ge(wm, 1)
            g.dma_gather(scr[:], w[:], woff[:], P, P, D, single_packet=False).then_inc(
                wu, 16
            )
            g.wait_ge(idx_sem, 16)
            g.dma_gather(
                emb[:], w[:], idx_t[:], TPC, TPC, D, single_packet=False
            ).then_inc(g0, 16)

        @block.sync
        def _(s):
            s.wait_ge(g0, 16)
            s.dma_start(out=out[0:P, :], in_=emb[:, 0, :]).then_inc(s0, 16)
            s.dma_start(out=out[3 * P : 4 * P, :], in_=emb[:, 3, :]).then_inc(s0, 16)

        @block.scalar
        def _(a):
            a.wait_ge(g0, 16)
            a.dma_start(out=out[P : 2 * P, :], in_=emb[:, 1, :]).then_inc(s1, 16)
            a.dma_start(out=out[2 * P : 3 * P, :], in_=emb[:, 2, :]).then_inc(s1, 16)
            # block-end DRAIN on each engine waits for its HWDGE queue
            # completion (verified exact on HW by the v1 baseline)

    return nc


def _pack_idx(flat_slice: np.ndarray) -> np.ndarray:
    """[TPC] int -> [128, 32] int16: idx i at (i%16, i//16), tiled 8x down
    the partitions for the 8 Q7 core groups."""
    blk = np.ascontiguousarray(flat_slice.astype(np.int16).reshape(IDX_COLS, 16).T)
    return np.ascontiguousarray(np.tile(blk, (8, 1)))


def kernel(x: np.ndarray, weight: np.ndarray) -> np.ndarray:
    import ml_dtypes

    x = np.asarray(x)
    flat = np.ascontiguousarray(x.reshape(-1)).astype(np.int64)
    w16 = np.ascontiguousarray(
        np.asarray(weight, dtype=np.float32).astype(ml_dtypes.bfloat16)
    )
    _CACHE["w16"] = w16  # test.py --profile reuses the converted table

    if "nc" not in _CACHE:
        nc = _build()
        # run_bass_via_pjrt binds the exec primitive directly and never
        # finalizes; Bacc needs finalize() -> compile() for register
        # allocation and the mlp-library MODIFY_POOL_CONFIG lowering
        nc.finalize()
        _CACHE["nc"] = nc
    nc = _CACHE["nc"]

    in_maps = [
        {
            "idx": _pack_idx(flat[i * TPC : (i + 1) * TPC]),
            "weight": w16,
        }
        for i in range(N_CORES)
    ]
    res = run_bass_kernel_spmd(nc, in_maps, list(range(N_CORES)))
    outs = [
        np.asarray(res.results[i]["out"]).astype(np.float32) for i in range(N_CORES)
    ]
    return np.concatenate(outs, axis=0).reshape(B, S, D)


# revision 14
# speedup vs baseline: 1.5132x; 1.4669x over previous
"""Embedding lookup (gather) kernel for Trainium2, 8 NeuronCores.

Reference computes emb[b,s,:] = weight[x[b,s],:]. Data-parallel over the
B*S = 4096 tokens, 512 tokens per core. The [32000, 512] f32 table is
converted to bf16 on the host (rel err ~2^-8, far inside the 2e-2 gate),
halving HBM traffic in both directions versus the f32 v1 baseline.

The HW SWDGE consumes exactly ONE row-offset per SBUF partition per
indirect DMA, so 512 rows take four 128-offset instructions (~994ns fixed
+ ~0.34ns/desc each), serialized on gpsimd. Rejected alternatives, both
measured on HW:
  - one InstDMAGatherAnt for all 512 rows: its Q7 loop runs ~5-7ns/idx
    AND needs the mlp ucode library whose ~9us IRAM load is paid inside
    EVERY profiled execution (30.3us total vs 19.7).
  - chunk 3's store on the gathers' SWDGE queue with no sem (relying on
    per-engine FIFO order): the SDMA engine pipelines the store's SBUF
    reads ahead of the gather's landing writes -> NaNs in chunk 3, and
    the SWDGE completion receipt made it no faster anyway.

v4: the warmup gather (pays SWDGE first-instruction overhead while the
idx DMA is in flight) reads its zero offsets from the framework's
const-fp32-0.0 tensor (memset by the bass preamble BEFORE the engine
barrier) bitcast to int32 - the v1 memset+sem chain (~1.1us) is gone and
the warmup issues as gpsimd's first body instruction.

Token layout per core is j-major: idx[p, j] = token j*128+p, gathered row
(p, j) sits at emb[p, j*D:(j+1)*D], each 128-row store is one contiguous
128KiB block, and the host-side unshard is a plain reshape.
"""

import numpy as np

import concourse.bass as bass
from concourse import mybir
from concourse.bass_utils import run_bass_kernel_spmd

B, S = 4, 1024
V, D = 32000, 512
N_CORES = 8
TOK = B * S                      # 4096 total tokens
TPC = TOK // N_CORES             # 512 tokens per core
P = 128                          # SBUF partitions
NCH = TPC // P                   # 4 j-slots of 128 rows

_CACHE: dict = {}


def _build() -> bass.Bass:
    nc = bass.Bass()
    idx = nc.dram_tensor("idx", [P, NCH], mybir.dt.int32, kind="ExternalInput")
    w = nc.dram_tensor("weight", [V, D], mybir.dt.bfloat16, kind="ExternalInput")
    out = nc.dram_tensor("out", [TPC, D], mybir.dt.bfloat16, kind="ExternalOutput")
    zero_off = nc.const_aps.aps[(mybir.dt.float32, 0.0)].bitcast(mybir.dt.int32)
    with (
        nc.Block() as block,
        nc.semaphore("idx_sem") as idx_sem,
        nc.semaphore("g0") as g0,
        nc.semaphore("g1") as g1,
        nc.semaphore("g2") as g2,
        nc.semaphore("g3") as g3,
        nc.semaphore("wu") as wu,
        nc.semaphore("s0") as s0,
        nc.semaphore("s1") as s1,
        nc.sbuf_tensor("idx_t", [P, NCH], mybir.dt.int32) as idx_t,
        nc.sbuf_tensor("emb", [P, NCH * D], mybir.dt.bfloat16) as emb,
        nc.sbuf_tensor("scr", [P, D], mybir.dt.bfloat16) as scr,
    ):
        gsems = [g0, g1, g2, g3]

        @block.sync
        def _(s):
            s.dma_start(out=idx_t[:], in_=idx[:]).then_inc(idx_sem, 16)

        @block.gpsimd
        def _(g):
            # warm the SWDGE ring with a row-0 gather while the idx DMA is
            # in flight — offsets come from the preamble's const-zero
            # tensor, so this is gpsimd's first body instruction
            g.indirect_dma_start(
                out=scr[:],
                out_offset=None,
                in_=w[:],
                in_offset=bass.IndirectOffsetOnAxis(ap=zero_off[:, :1], axis=0),
            ).then_inc(wu, 16)
            g.wait_ge(idx_sem, 16)
            for j in range(NCH):
                g.indirect_dma_start(
                    out=emb[:, j * D : (j + 1) * D],
                    out_offset=None,
                    in_=w[:],
                    in_offset=bass.IndirectOffsetOnAxis(ap=idx_t[:, j : j + 1], axis=0),
                ).then_inc(gsems[j], 16)

        @block.sync
        def _(s):
            # sync's dispatch-after-wait is ~0.3us faster than scalar's, so
            # it takes the critical last chunk (j=3)
            s.wait_ge(g0, 16)
            s.dma_start(out=out[0:P, :], in_=emb[:, 0:D]).then_inc(s0, 16)
            s.wait_ge(g3, 16)
            s.dma_start(out=out[3 * P : 4 * P, :], in_=emb[:, 3 * D : 4 * D]).then_inc(
                s0, 16
            )

        @block.scalar
        def _(a):
            a.wait_ge(g1, 16)
            a.dma_start(out=out[P : 2 * P, :], in_=emb[:, D : 2 * D]).then_inc(s1, 16)
            a.wait_ge(g2, 16)
            a.dma_start(out=out[2 * P : 3 * P, :], in_=emb[:, 2 * D : 3 * D]).then_inc(
                s1, 16
            )
            # block-end DRAIN on each engine waits for its HWDGE queue
            # completion (verified exact on HW by the v1 baseline)

    return nc


def _pack_idx(flat_slice: np.ndarray) -> np.ndarray:
    """[TPC] int -> [128, 4] int32 j-major: idx[p, j] = token j*128+p."""
    return np.ascontiguousarray(flat_slice.astype(np.int32).reshape(NCH, P).T)


def kernel(x: np.ndarray, weight: np.ndarray) -> np.ndarray:
    import ml_dtypes

    x = np.asarray(x)
    flat = np.ascontiguousarray(x.reshape(-1)).astype(np.int64)
    w16 = np.ascontiguousarray(
        np.asarray(weight, dtype=np.float32).astype(ml_dtypes.bfloat16)
    )
    _CACHE["w16"] = w16  # test.py --profile reuses the converted table

    if "nc" not in _CACHE:
        _CACHE["nc"] = _build()
    nc = _CACHE["nc"]

    in_maps = [
        {
            "idx": _pack_idx(flat[i * TPC : (i + 1) * TPC]),
            "weight": w16,
        }
        for i in range(N_CORES)
    ]
    res = run_bass_kernel_spmd(nc, in_maps, list(range(N_CORES)))
    outs = [
        np.asarray(res.results[i]["out"]).astype(np.float32) for i in range(N_CORES)
    ]
    return np.concatenate(outs, axis=0).reshape(B, S, D)
